# revision 33
# baseline (speedup 1.0000x reference)
"""Trainium2 Bass kernel for nn_ARModel (AR(12) self-feeding recurrence).

Math: the reference scan is affine-linear in its initial history window
h0 = x[:, T-p:, :, 0] (the only part of x the output depends on):

    out[b, t, n] = sum_k W[t, n, k] * h0[b, n, k] + c[t, n]

where W (impulse-response coefficients) and c (bias response) depend only on
ar_params / bias and are unrolled on the host (weight preprocessing). This
removes the sequential T-scan from the device: the per-sample work becomes a
batch of tiny per-node matmuls.

Device mapping (per core, N sharded 8-ways -> 128 nodes/core):
  - groups of 2 nodes; per group one TensorE matmul (bf16 operands, f32 PSUM)
        out[64*i + b, t] = sum_{i,k} S[13*i + k, 64*i + b] * M[13*i + k, t]
    with S = block-diagonal h0 (plus a row of ones for the bias term) as the
    stationary operand and M = W rows (plus the c row) as the moving operand.
  - 4 groups live on partition strips {0,32,64,96} so DMA uses all 128
    partitions and the 4 strip matmuls run concurrently in separate PE
    row-groups (tile_position).
  - raw (non-Tile) pipeline, hand-rolled semaphores: input DMAs on the ACT
    HWDGE ring, 64 matmuls into two double-buffered 2-bank PSUM chains,
    drained by DVE (strips 0-1) and ACT (strips 2-3) into per-j SBUF staging
    slots, output DMAs on the SP HWDGE ring. The output stream saturates
    HBM (~342 GB/s of the ~358 GB/s per-core cap).
  - measured ~35.3us HW exec per core (neuron-profile "useful time"),
    vs ~27.5us for the pure 9.4MB output stream.

Hardware gotchas encoded below:
  - semaphores are NOT cleared at NEFF entry without target_bir_lowering;
    stale values from a previous execution make every wait pass early ->
    sem_clear at kernel end behind a sem-only all-engine barrier.
  - LDWEIGHTS pull-ahead can race the same row-group's in-flight matmul
    when the PE queue runs hot -> serialize consecutive j's on the PE.
"""

import numpy as np

B, T, N, P = 64, 288, 1024, 12
NCORES = 8
NPC = N // NCORES  # nodes per core = 128
K = P + 1          # contraction rows per node (12 coeffs + 1 bias row)
JBLK = 16          # j index: 16 column blocks
STRIPS = 4         # partition strips at 0/32/64/96
GROUPS = JBLK * STRIPS          # 64 groups of 2 nodes per core
CHUNK_G = 8                     # groups per output DMA chunk
NCHUNK = GROUPS // CHUNK_G      # 8 chunks

_compiled = {}


def _build_bass():
    """Raw (non-Tile) Bacc kernel with hand-rolled semaphores.

    Streams:
      Scalar : 4 input DMAs (qActDynamicHW ring), then ACT copies of PSUM
               strips 2-3 per j
      Tensor : per j, 4 concurrent strip matmuls into two 2-bank PSUM
               tensors (each double-buffered j%2)
      Vector : DVE copies of PSUM strips 0-1 per j
      Sync   : 16 output DMAs (qSPDynamicHW ring), one per j, each from its
               own SBUF staging slot
    """
    import concourse.mybir as mybir
    from concourse import bacc

    f32 = mybir.dt.float32
    bf16 = mybir.dt.bfloat16
    nc = bacc.Bacc("TRN2", target_bir_lowering=False)

    JW = 128 + T  # columns per j-block in the combined input: S (128) + M (288)
    i_d = nc.dram_tensor("inp", (128, JBLK * JW), bf16, kind="ExternalInput")
    o_d = nc.dram_tensor("out", (128, GROUPS * T), f32, kind="ExternalOutput")

    # input DMA chunks (j ranges); first small so compute starts early
    chunks = [(0, 1), (1, 6), (6, 11), (11, 16)]

    def chunk_idx(j):
        for ci, (j0, j1) in enumerate(chunks):
            if j0 <= j < j1:
                return ci
        raise AssertionError

    in_sb = nc.alloc_sbuf_tensor("in_sb", [128, JBLK * JW], bf16).ap()
    # one staging slot per j: no within-execution slot reuse, so no
    # DMA-completion semaphores are needed at all (16 x 4.6KB/partition)
    och = [
        nc.alloc_sbuf_tensor(f"och{i}", [128, STRIPS * T], f32).ap()
        for i in range(JBLK)
    ]
    # two independent double-buffered PSUM chains: strips 0-1 drained by DVE,
    # strips 2-3 drained by ACT
    psv = [nc.alloc_psum_tensor(f"psv{i}", [128, 2, 512], f32).ap() for i in range(2)]
    pss = [nc.alloc_psum_tensor(f"pss{i}", [128, 2, 512], f32).ap() for i in range(2)]

    sem_in = [nc.alloc_semaphore(f"sem_in{c}") for c in range(len(chunks))]
    sem_mmv = nc.alloc_semaphore("sem_mmv")
    sem_mms = nc.alloc_semaphore("sem_mms")
    sem_cpv = nc.alloc_semaphore("sem_cpv")
    sem_cps = nc.alloc_semaphore("sem_cps")
    sem_done = nc.alloc_semaphore("sem_done")
    # completion counter for output DMAs; required by the framework but has
    # no waiters, so its cross-execution accumulation is harmless
    sem_junk = nc.alloc_semaphore("sem_junk")

    HALF = 2 * T  # columns one engine copies per j (2 strips x 288)

    def copy_stream(eng, ps2, lo_strip, sem_mm, sem_cp, copy_fn):
        for j in range(JBLK):
            eng.wait_ge(sem_mm, j + 1)
            dst0 = lo_strip * T
            copy_fn(
                och[j][:, dst0 : dst0 + HALF],
                ps2[j % 2][:, :, :T],
            ).then_inc(sem_cp, 1)

    with nc.Block() as block:

        @block.scalar
        def _(eng):
            for c, (j0, j1) in enumerate(chunks):
                eng.dma_start(
                    in_sb[:, j0 * JW : j1 * JW], i_d[:, j0 * JW : j1 * JW]
                ).then_inc(sem_in[c], 16)
            copy_stream(eng, pss, 2, sem_mms, sem_cps, nc.scalar.copy)

        @block.vector
        def _(eng):
            copy_stream(eng, psv, 0, sem_mmv, sem_cpv, nc.vector.tensor_copy)

        @block.tensor
        def _(eng):
            for j in range(JBLK):
                ci = chunk_idx(j)
                if j == chunks[ci][0]:
                    eng.wait_ge(sem_in[ci], 16)
                if j >= 1:
                    # serialize against previous j's matmuls: LDWEIGHTS
                    # pull-ahead must not race the same row-group's
                    # in-flight matmul (drops first-exec corruption)
                    eng.wait_ge(sem_mms, j)
                for half, (ps2, sem_cp, sem_mm) in enumerate(
                    [(psv, sem_cpv, sem_mmv), (pss, sem_cps, sem_mms)]
                ):
                    if j >= 2:
                        eng.wait_ge(sem_cp, j - 1)
                    for ds in range(2):
                        s = 2 * half + ds
                        mm = nc.tensor.matmul(
                            ps2[j % 2][:, ds, :T],
                            in_sb[32 * s : 32 * s + 2 * K, j * JW : j * JW + 128],
                            in_sb[32 * s : 32 * s + 2 * K, j * JW + 128 : (j + 1) * JW],
                            start=True,
                            stop=True,
                            tile_position=(32 * s, 0),
                        )
                        if ds == 1:
                            mm.then_inc(sem_mm, 1)

        @block.sync
        def _(eng):
            cols = STRIPS * T
            for j in range(JBLK):
                eng.wait_ge(sem_cpv, j + 1)
                eng.wait_ge(sem_cps, j + 1)
                eng.dma_start(o_d[:, j * cols : (j + 1) * cols], och[j]).then_inc(
                    sem_junk, 16
                )
            # all waiters of the pipeline sems have executed once this issues
            eng.sem_inc(sem_done, 1)

        @block.gpsimd
        def _(eng):
            # join the pipeline end: Sync has issued its last DMA (this does
            # NOT wait for DMA completion, which would drag the exit barrier
            # past the final transfer)
            eng.wait_ge(sem_done, 1)

    # Reset our semaphores for the next execution of this NEFF (Bass only
    # emits an entry sem_clear under target_bir_lowering). The sem-only
    # barrier orders every engine past its last pipeline wait first.
    nc.all_engine_barrier(sem_only=True)
    from concourse.bass import compact_to_ranges

    nums = [s.num for s in sem_in + [sem_mmv, sem_mms, sem_cpv, sem_cps, sem_done]]
    for r in compact_to_ranges(nums):
        nc.gpsimd.sem_clear(r)

    nc.finalize()
    return nc


def _unroll_weights(ar_params, bias):
    """Impulse-response unroll: W[t, n, k] = d s_t / d h0[k], c[t, n] = bias part."""
    a = ar_params.astype(np.float64)
    Wfull = np.zeros((T + P, N, P), np.float64)
    Wfull[np.arange(P), :, np.arange(P)] = 1.0
    c = np.zeros((T + P, N), np.float64)
    b64 = bias.astype(np.float64)
    for t in range(T):
        Wfull[P + t] = np.einsum("nj,jnk->nk", a, Wfull[t : t + P])
        c[P + t] = np.einsum("nj,jn->n", a, c[t : t + P]) + b64
    return Wfull[P:].astype(np.float32), c[P:].astype(np.float32)


def _pack_core(h0c, Wc, cc):
    """Build per-core DMA images.

    h0c: (B, P, 128)   last-P x slice for this core's nodes  [b, k, nl]
    Wc:  (T, 128, P)   [t, nl, k]
    cc:  (T, 128)      [t, nl]
    node index nl = 8*j + 2*s + i  (j in 0..15, s strip 0..3, i 0..1)
    """
    # moving operand: M[s, 13*i + k, j, t]
    Wr = Wc.transpose(1, 2, 0).reshape(JBLK, STRIPS, 2, P, T)  # (j, s, i, k, t)
    M = np.zeros((STRIPS, 2, K, JBLK, T), np.float32)
    M[:, :, :P] = Wr.transpose(1, 2, 3, 0, 4)
    ccr = cc.T.reshape(JBLK, STRIPS, 2, T)  # (j, s, i, t)
    M[:, :, P] = ccr.transpose(1, 2, 0, 3)
    m_pack = np.zeros((STRIPS, 32, JBLK, T), np.float32)
    m_pack[:, : 2 * K] = M.reshape(STRIPS, 2 * K, JBLK, T)

    # stationary operand: S[s, 13*i + k, j, 64*i + b] block-diagonal in i
    h0r = h0c.transpose(2, 1, 0).reshape(JBLK, STRIPS, 2, P, B)  # (j, s, i, k, b)
    S = np.zeros((STRIPS, 2, K, JBLK, 2, B), np.float32)
    hsk = h0r.transpose(1, 2, 3, 0, 4)  # (s, i, k, j, b)
    for i in range(2):
        S[:, i, :P, :, i, :] = hsk[:, i]
        S[:, i, P, :, i, :] = 1.0
    s_pack = np.zeros((STRIPS, 32, JBLK, 2 * B), np.float32)
    s_pack[:, : 2 * K] = S.reshape(STRIPS, 2 * K, JBLK, 2 * B)

    # combined per-j layout: [S_j (128 cols) | M_j (288 cols)]
    inp = np.concatenate([s_pack, m_pack], axis=3)  # (4, 32, 16, 416)
    import ml_dtypes

    return np.ascontiguousarray(inp).reshape(128, JBLK * (128 + T)).astype(
        ml_dtypes.bfloat16
    )


def kernel(x, ar_params, bias):
    from concourse import bass_utils

    x = np.ascontiguousarray(np.asarray(x, dtype=np.float32))
    ar_params = np.asarray(ar_params, dtype=np.float32)
    bias = np.asarray(bias, dtype=np.float32)

    W, c = _unroll_weights(ar_params, bias)
    h0 = x[:, T - P :, :, 0]  # (B, P, N)

    in_maps = []
    for ci in range(NCORES):
        sl = slice(ci * NPC, (ci + 1) * NPC)
        inp = _pack_core(h0[:, :, sl], W[:, sl, :], c[:, sl])
        in_maps.append({"inp": inp})

    if "nc" not in _compiled:
        _compiled["nc"] = _build_bass()
    res = bass_utils.run_bass_kernel_spmd(
        _compiled["nc"], in_maps, core_ids=list(range(NCORES))
    )
    _compiled["last_result"] = res  # exec_time_ns etc. when BASS_TRACE=1

    full = np.empty((B, T, N), np.float32)
    for ci in range(NCORES):
        r = res.results[ci]["out"].reshape(2, B, GROUPS, T)  # (i, b, g, t)
        blk = np.transpose(r, (1, 3, 2, 0))  # (b, t, g, i); nl = 2*g + i
        full[:, :, ci * NPC : (ci + 1) * NPC] = blk.reshape(B, T, NPC)
    return full[..., None]
